# revision 1
# baseline (speedup 1.0000x reference)
"""Conv1d  x[32,256,4096] * W[512,256,9] + b  (stride 1, pad 4)  -> [32,512,4096]
on 8 TRN2 NeuronCores, data-parallel over the batch dim (4 batches/core).

Per core the conv is computed as PSUM-accumulated matmuls:
    out[o, p] = bias[o] + sum_{k-tile, tap} W_t[c, o].T-applied  @  xpad[c, p+t]
Each [128 outch x 512 pos] output tile accumulates 2 k-tiles x 9 taps = 18
matmuls.  The padded x rows ([128, 4104]) live whole in SBUF, so the rhs for
tap t is just an offset slice - no data movement between taps.  Weights are
pre-transposed on the host so each lhsT tile is a contiguous SBUF slice.
Matmuls run in float32r (full PE rate at N=512, ~TF32 precision).
"""

import os

import numpy as np

B, CIN, COUT, KW, L, PAD = 32, 256, 512, 9, 4096, 4
NCORES = 8
BPC = B // NCORES  # batches per core
LP = L + 2 * PAD  # padded length
KT = CIN // 128  # contraction k-tiles
MT = COUT // 128  # output-channel tiles
NFREE = 512  # matmul moving free dim
NT = L // NFREE  # output-position tiles

_CACHE = {}


def _split_multi_waits(nc, max_waits=1):
    # This container's walrus accepts at most one sync wait per instruction;
    # TileContext's tail drain carries several.  Hoist the excess onto
    # same-engine EventSemaphore instructions inserted just before it.
    import concourse.mybir as mybir

    for fn in nc.m.functions:
        for bb in fn.blocks:
            new_list = []
            changed = False
            for ins in bb.instructions:
                si = ins.sync_info
                if si is not None and si.on_wait and len(si.on_wait) > max_waits:
                    waits = list(si.on_wait)
                    hoist, keep = waits[:-max_waits], waits[-max_waits:]
                    for j, w in enumerate(hoist):
                        ev = mybir.InstEventSemaphore(
                            name=f"{ins.name}_wsplit{j}",
                            engine=ins.engine,
                            ins=[],
                            outs=[],
                            sync_info=mybir.SyncInfo(on_wait=[w], on_update=[]),
                        )
                        new_list.append(ev)
                    ins.sync_info = mybir.SyncInfo(
                        on_wait=keep, on_update=list(si.on_update)
                    )
                    changed = True
                new_list.append(ins)
            if changed:
                bb.instructions = new_list


def _build(reps=1, timing=False):
    """Build the per-core Bass module.

    timing=True makes the big I/O tensors Internal DRAM (nothing shipped
    through the axon tunnel) and wraps the whole body in a hardware For_i
    loop of `reps` iterations, so on-device time dominates wall clock."""
    import concourse.bass as bass
    import concourse.mybir as mybir
    import concourse.tile as tile

    f32 = mybir.dt.float32
    f32r = mybir.dt.float32r

    nc = bass.Bass()
    big_kind = "Internal" if timing else "ExternalInput"
    xp = nc.dram_tensor("xp", [BPC, CIN, LP], f32r, kind=big_kind)
    w = nc.dram_tensor("w", [KT, 128, KW, COUT], f32r, kind="ExternalInput")
    bias = nc.dram_tensor("bias", [128, MT], f32, kind="ExternalInput")
    out = nc.dram_tensor(
        "out", [BPC, COUT, L], f32, kind="Internal" if timing else "ExternalOutput"
    )
    done = (
        nc.dram_tensor("done", [1, 1], f32, kind="ExternalOutput") if timing else None
    )

    with tile.TileContext(nc) as tc:
        with (
            tc.tile_pool(name="wpool", bufs=1) as wpool,
            tc.tile_pool(name="xpool", bufs=2) as xpool,
            tc.tile_pool(name="opool", bufs=8) as opool,
            tc.tile_pool(name="psum", bufs=8, space="PSUM") as ppool,
        ):
            # One tile per (k-tile, tap): the first matmul only has to wait
            # for a 256 KB slice, not the whole 4.6 MB weight block (sim
            # showed a 21.7 us PE stall at startup with one big tile).
            # Tap-0 tiles are DMA'd first - they gate the first matmuls.
            wk = [[None] * KW for _ in range(KT)]
            for t in range(KW):
                for k in range(KT):
                    t_ = wpool.tile([128, COUT], f32r, name="wkt", tag=f"w{k}_{t}")
                    nc.sync.dma_start(t_[:], w[k, :, t, :])
                    wk[k][t] = t_
            bias_sb = wpool.tile([128, MT], f32, tag="bias")
            nc.sync.dma_start(bias_sb[:], bias[:, :])

            # x loads are chunked 4-way along positions (8-col tap halo per
            # chunk) so the 8 per-batch DMAs spread across the HWDGE queues:
            # the first matmul waits on one 528 KB chunk, not 4.2 MB on one
            # queue (the sim's 21.7 us startup PE stall).
            CW = LP // 4 + 6  # 1032: two 512-blocks + 8-tap halo

            def body(_iv=None):
                for b in range(BPC):
                    xb = [[None] * 4 for _ in range(KT)]
                    for c in range(4):
                        for k in range(KT):
                            t_ = xpool.tile(
                                [128, CW], f32r, name="xb", tag=f"x{k}c{c}"
                            )
                            # SWDGE keeps x prefetch off the HWDGE path that
                            # carries weight loads and output stores.
                            nc.gpsimd.dma_start(
                                t_[:],
                                xp[
                                    b,
                                    k * 128 : (k + 1) * 128,
                                    c * 1024 : c * 1024 + CW,
                                ],
                            )
                            xb[k][c] = t_
                    for m in range(MT):
                        pts = [
                            ppool.tile([128, NFREE], f32, name="pt", tag="pt")
                            for _ in range(NT)
                        ]
                        first = True
                        for k in range(KT):
                            for t in range(KW):
                                lhsT = wk[k][t][:, m * 128 : (m + 1) * 128]
                                last = k == KT - 1 and t == KW - 1
                                for j in range(NT):
                                    lo = (j % 2) * NFREE + t
                                    nc.tensor.matmul(
                                        pts[j][:, :],
                                        lhsT,
                                        xb[k][j // 2][:, lo : lo + NFREE],
                                        start=first,
                                        stop=last,
                                    )
                                first = False
                        # Alternate the psum->SBUF bias-add copy between ACT
                        # and DVE: halves the tail drain after the last
                        # matmul and frees PSUM banks sooner at m boundaries.
                        for j in range(NT):
                            ot = opool.tile([128, NFREE], f32, name="ot", tag="ot")
                            if j % 2 == 0:
                                nc.scalar.add(
                                    ot[:, :], pts[j][:, :], bias_sb[:, m : m + 1]
                                )
                            else:
                                nc.vector.tensor_scalar_add(
                                    ot[:, :], pts[j][:, :], bias_sb[:, m : m + 1]
                                )
                            nc.sync.dma_start(
                                out[
                                    b,
                                    m * 128 : (m + 1) * 128,
                                    j * NFREE : (j + 1) * NFREE,
                                ],
                                ot[:, :],
                            )

            if timing and reps > 1:
                with tc.For_i(0, reps, 1):
                    body()
            else:
                body()

            if timing:
                dt_sb = opool.tile([128, 1], f32, name="dt_sb", tag="dt")
                nc.vector.memset(dt_sb[:, :], 0.0)
                nc.sync.dma_start(done[:, :], dt_sb[0:1, :])

    _split_multi_waits(nc)
    return nc


def kernel(x=None, weights=None, bias=None):
    from concourse.bass_utils import run_bass_kernel_spmd

    x = np.ascontiguousarray(np.asarray(x), dtype=np.float32)
    W = np.ascontiguousarray(np.asarray(weights), dtype=np.float32)
    bv = np.asarray(bias, dtype=np.float32)

    xpad = np.zeros((B, CIN, LP), np.float32)
    xpad[:, :, PAD : PAD + L] = x
    # w_arr[k, c, t, o] = W[o, k*128+c, t]  -> lhsT tiles are contiguous slices
    w_arr = np.ascontiguousarray(W.transpose(1, 2, 0).reshape(KT, 128, KW, COUT))
    bias_r = np.ascontiguousarray(bv.reshape(MT, 128).T)

    nc = _CACHE.get("nc")
    if nc is None:
        nc = _CACHE["nc"] = _build()

    in_maps = [
        {
            "xp": np.ascontiguousarray(xpad[c * BPC : (c + 1) * BPC]),
            "w": w_arr,
            "bias": bias_r,
        }
        for c in range(NCORES)
    ]
    # NTFF profiling needs an axon hook this container lacks; make sure a
    # stray BASS_TRACE in the environment cannot route us into that path.
    os.environ["BASS_NEVER_TRACE"] = "1"
    res = run_bass_kernel_spmd(nc, in_maps, core_ids=list(range(NCORES)))
    kernel.last_results = res
    results = res.results
    return np.concatenate([results[c]["out"] for c in range(NCORES)], axis=0)


kernel.last_results = None



# revision 4
# speedup vs baseline: 1.1277x; 1.1277x over previous
"""Conv1d  x[32,256,4096] * W[512,256,9] + b  (stride 1, pad 4)  -> [32,512,4096]
on 8 TRN2 NeuronCores, data-parallel over the batch dim (4 batches/core).

Per core the conv is computed as PSUM-accumulated matmuls:
    out[o, p] = bias[o] + sum_{k-tile, tap} W_t[c, o].T-applied  @  xpad[c, p+t]
Each [128 outch x 512 pos] output tile accumulates 2 k-tiles x 9 taps = 18
matmuls.  The padded x rows ([128, 4104]) live whole in SBUF, so the rhs for
tap t is just an offset slice - no data movement between taps.  Weights are
pre-transposed on the host so each lhsT tile is a contiguous SBUF slice.

Inputs are bf16: same PE rate as float32r (1 row/cycle at N=512) but half
the SBUF moving-read bytes, half the stationary-load bytes and half the HBM
traffic for x and w.  The 2304-term contraction accumulates in fp32 PSUM;
bf16 input rounding gives ~2e-3 rel err vs the 2e-2 gate.
"""

import os

import numpy as np

B, CIN, COUT, KW, L, PAD = 32, 256, 512, 9, 4096, 4
NCORES = 8
BPC = B // NCORES  # batches per core
LP = L + 2 * PAD  # padded length
KT = CIN // 128  # contraction k-tiles
MT = COUT // 128  # output-channel tiles
NFREE = 512  # matmul moving free dim
NT = L // NFREE  # output-position tiles

_CACHE = {}


def _split_multi_waits(nc, max_waits=1):
    # This container's walrus accepts at most one sync wait per instruction;
    # TileContext's tail drain carries several.  Hoist the excess onto
    # same-engine EventSemaphore instructions inserted just before it.
    import concourse.mybir as mybir

    for fn in nc.m.functions:
        for bb in fn.blocks:
            new_list = []
            changed = False
            for ins in bb.instructions:
                si = ins.sync_info
                if si is not None and si.on_wait and len(si.on_wait) > max_waits:
                    waits = list(si.on_wait)
                    hoist, keep = waits[:-max_waits], waits[-max_waits:]
                    for j, w in enumerate(hoist):
                        ev = mybir.InstEventSemaphore(
                            name=f"{ins.name}_wsplit{j}",
                            engine=ins.engine,
                            ins=[],
                            outs=[],
                            sync_info=mybir.SyncInfo(on_wait=[w], on_update=[]),
                        )
                        new_list.append(ev)
                    ins.sync_info = mybir.SyncInfo(
                        on_wait=keep, on_update=list(si.on_update)
                    )
                    changed = True
                new_list.append(ins)
            if changed:
                bb.instructions = new_list


def _build(reps=1, timing=False):
    """Build the per-core Bass module.

    timing=True makes the big I/O tensors Internal DRAM (nothing shipped
    through the axon tunnel) and wraps the whole body in a hardware For_i
    loop of `reps` iterations, so on-device time dominates wall clock."""
    import concourse.bass as bass
    import concourse.mybir as mybir
    import concourse.tile as tile

    f32 = mybir.dt.float32
    bf16 = mybir.dt.bfloat16

    nc = bass.Bass()
    big_kind = "Internal" if timing else "ExternalInput"
    xp = nc.dram_tensor("xp", [BPC, CIN, LP], bf16, kind=big_kind)
    w = nc.dram_tensor("w", [KT, 128, KW, COUT], bf16, kind="ExternalInput")
    bias = nc.dram_tensor("bias", [128, MT], f32, kind="ExternalInput")
    out = nc.dram_tensor(
        "out", [BPC, COUT, L], f32, kind="Internal" if timing else "ExternalOutput"
    )
    done = (
        nc.dram_tensor("done", [1, 1], f32, kind="ExternalOutput") if timing else None
    )

    with tile.TileContext(nc) as tc:
        with (
            tc.tile_pool(name="wpool", bufs=1) as wpool,
            tc.tile_pool(name="xpool", bufs=2) as xpool,
            tc.tile_pool(name="opool", bufs=8) as opool,
            tc.tile_pool(name="psum", bufs=8, space="PSUM") as ppool,
        ):
            # One tile per (k-tile, tap): the first matmul only has to wait
            # for a small slice, not the whole weight block.  Tap-0 tiles are
            # DMA'd first - they gate the first matmuls.
            wk = [[None] * KW for _ in range(KT)]
            for t in range(KW):
                for k in range(KT):
                    t_ = wpool.tile([128, COUT], bf16, name="wkt", tag=f"w{k}_{t}")
                    nc.sync.dma_start(t_[:], w[k, :, t, :])
                    wk[k][t] = t_
            bias_sb = wpool.tile([128, MT], f32, tag="bias")
            nc.sync.dma_start(bias_sb[:], bias[:, :])

            # x loads are chunked 4-way along positions (8-col tap halo per
            # chunk) so per-batch DMAs spread across queues and the first
            # matmul gates on one chunk only.  They ride the ACT HWDGE queue:
            # off the SP queue that carries weight loads and output stores,
            # and SWDGE (gpsimd) trips walrus codegen inside For_i loops.
            CW = LP // 4 + 6  # 1032: two 512-blocks + 8-tap halo

            def body(_iv=None):
                for b in range(BPC):
                    xb = [[None] * 4 for _ in range(KT)]
                    for c in range(4):
                        for k in range(KT):
                            t_ = xpool.tile(
                                [128, CW], bf16, name="xb", tag=f"x{k}c{c}"
                            )
                            nc.scalar.dma_start(
                                t_[:],
                                xp[
                                    b,
                                    k * 128 : (k + 1) * 128,
                                    c * 1024 : c * 1024 + CW,
                                ],
                            )
                            xb[k][c] = t_
                    for m in range(MT):
                        pts = [
                            ppool.tile([128, NFREE], f32, name="pt", tag="pt")
                            for _ in range(NT)
                        ]
                        first = True
                        for k in range(KT):
                            for t in range(KW):
                                lhsT = wk[k][t][:, m * 128 : (m + 1) * 128]
                                last = k == KT - 1 and t == KW - 1
                                for j in range(NT):
                                    lo = (j % 2) * NFREE + t
                                    nc.tensor.matmul(
                                        pts[j][:, :],
                                        lhsT,
                                        xb[k][j // 2][:, lo : lo + NFREE],
                                        start=first,
                                        stop=last,
                                    )
                                first = False
                        # Alternate the psum->SBUF bias-add copy between ACT
                        # and DVE: halves the tail drain after the last
                        # matmul and frees PSUM banks sooner at m boundaries.
                        for j in range(NT):
                            ot = opool.tile([128, NFREE], f32, name="ot", tag="ot")
                            if j % 2 == 0:
                                nc.scalar.add(
                                    ot[:, :], pts[j][:, :], bias_sb[:, m : m + 1]
                                )
                            else:
                                nc.vector.tensor_scalar_add(
                                    ot[:, :], pts[j][:, :], bias_sb[:, m : m + 1]
                                )
                            nc.sync.dma_start(
                                out[
                                    b,
                                    m * 128 : (m + 1) * 128,
                                    j * NFREE : (j + 1) * NFREE,
                                ],
                                ot[:, :],
                            )

            if timing and reps > 1:
                with tc.For_i(0, reps, 1):
                    body()
            else:
                body()

            if timing:
                dt_sb = opool.tile([128, 1], f32, name="dt_sb", tag="dt")
                nc.vector.memset(dt_sb[:, :], 0.0)
                nc.sync.dma_start(done[:, :], dt_sb[0:1, :])

    _split_multi_waits(nc)
    return nc


def kernel(x=None, weights=None, bias=None):
    import ml_dtypes
    from concourse.bass_utils import run_bass_kernel_spmd

    x = np.ascontiguousarray(np.asarray(x), dtype=np.float32)
    W = np.ascontiguousarray(np.asarray(weights), dtype=np.float32)
    bv = np.asarray(bias, dtype=np.float32)

    xpad = np.zeros((B, CIN, LP), ml_dtypes.bfloat16)
    xpad[:, :, PAD : PAD + L] = x.astype(ml_dtypes.bfloat16)
    # w_arr[k, c, t, o] = W[o, k*128+c, t]  -> lhsT tiles are contiguous slices
    w_arr = np.ascontiguousarray(
        W.transpose(1, 2, 0).reshape(KT, 128, KW, COUT).astype(ml_dtypes.bfloat16)
    )
    bias_r = np.ascontiguousarray(bv.reshape(MT, 128).T)

    nc = _CACHE.get("nc")
    if nc is None:
        nc = _CACHE["nc"] = _build()

    in_maps = [
        {
            "xp": np.ascontiguousarray(xpad[c * BPC : (c + 1) * BPC]),
            "w": w_arr,
            "bias": bias_r,
        }
        for c in range(NCORES)
    ]
    # NTFF profiling needs an axon hook this container lacks; make sure a
    # stray BASS_TRACE in the environment cannot route us into that path.
    os.environ["BASS_NEVER_TRACE"] = "1"
    res = run_bass_kernel_spmd(nc, in_maps, core_ids=list(range(NCORES)))
    kernel.last_results = res
    results = res.results
    return np.concatenate([results[c]["out"] for c in range(NCORES)], axis=0)


kernel.last_results = None
